# revision 45
# baseline (speedup 1.0000x reference)
"""Additive attention (Bahdanau) Trainium2 kernel, 8-core data parallel.

out = softmax_k(mask(sum_h w_v[h] * tanh(q@Wq [q,h] + k@Wk [k,h]))) @ V

Per-core work (2 batches): dominated by tanh over 2*64*512*256 = 16.8M
elements on the Scalar (ACT) engine -> ~110us floor.  Pipeline:
  DVE:  sum slab  s[h, q, k] = kfT[h,k] + qfT[h,q]   (bf16 tensor_scalar)
  ACT:  tanh over big slabs (several q's per instruction)
  PE :  score rows via accumulating one-hot matmuls
        lhsT_q = w_v (x) e_q  ->  psum[q, :] += w_v . tanh_feat_q
  then masked exp (bias rows from valid_lens fused into the psum
  evacuation, softmax denominator from the exp's accum_out), row
  normalization, PE-transpose of attn, attn.T stationary @ V.

Engines execute their streams in order, so emission order is tuned so the
first tanh slab issues early (critical DMAs first on two queues) and
batch 1's transposes/projections are emitted before batch 0's softmax.
"""

import os
from contextlib import ExitStack

import ml_dtypes
import numpy as np

import concourse.bacc as bacc
import concourse.mybir as mybir
import concourse.tile as tile
from concourse.bass_utils import run_bass_kernel_spmd

F32 = mybir.dt.float32
BF16 = mybir.dt.bfloat16
I32 = mybir.dt.int32
AF = mybir.ActivationFunctionType
ALU = mybir.AluOpType

B, NQ, NK, QS, KS, H, VD = 16, 64, 512, 256, 256, 256, 256
NCORES = 8
BPC = B // NCORES  # batches per core
MASK_NEG = -30.0  # exp(-30+5) ~ 1e-11 of any valid term; scores are in [-5, 5]

CHUNKS_B0 = [2, 2, 4, 8] + [12] * 4
CHUNKS_B1 = [12] * 4 + [8, 4, 4]


def _build():
    nc = bacc.Bacc()
    q_d = nc.declare_dram_parameter("queries", [BPC, NQ, QS], F32, isOutput=False)
    k_d = nc.declare_dram_parameter("keys", [BPC, NK, KS], F32, isOutput=False)
    v_d = nc.declare_dram_parameter("values", [BPC, NK, VD], F32, isOutput=False)
    vl_d = nc.declare_dram_parameter("valid_lens", [BPC, 1], I32, isOutput=False)
    wq_d = nc.declare_dram_parameter("W_q", [QS, H], F32, isOutput=False)
    wk_d = nc.declare_dram_parameter("W_k", [KS, H], F32, isOutput=False)
    wv_d = nc.declare_dram_parameter("w_v", [H], F32, isOutput=False)
    out_d = nc.declare_dram_parameter("out", [BPC, NQ, VD], F32, isOutput=True)

    # compile-time constants baked into the NEFF
    ident_d = nc.inline_tensor(np.eye(128, dtype=np.float32), name="ident_c")
    identb_d = nc.inline_tensor(
        np.eye(128).astype(ml_dtypes.bfloat16), name="identb_c"
    )
    # one-hot pattern for the score-reduction weights
    diag_d = nc.inline_tensor(
        np.eye(NQ, NQ).astype(ml_dtypes.bfloat16).reshape(NQ * NQ), name="diag_c"
    )
    krow_d = nc.inline_tensor(np.arange(NK, dtype=np.float32), name="krow_c")

    with ExitStack() as ctx:
        tc = ctx.enter_context(tile.TileContext(nc))
        consts = ctx.enter_context(tc.tile_pool(name="consts", bufs=1))
        setup = ctx.enter_context(tc.tile_pool(name="setup", bufs=2))
        slabs = ctx.enter_context(tc.tile_pool(name="slabs", bufs=2))
        sm = ctx.enter_context(tc.tile_pool(name="sm", bufs=1))
        outp = ctx.enter_context(tc.tile_pool(name="outp", bufs=2))
        ps_sc = ctx.enter_context(tc.tile_pool(name="ps_sc", bufs=2, space="PSUM"))
        ps_misc = ctx.enter_context(tc.tile_pool(name="ps_misc", bufs=2, space="PSUM"))
        ps_out = ctx.enter_context(tc.tile_pool(name="ps_out", bufs=2, space="PSUM"))

        # ---------------- loads (critical first, two queues) ----------------
        # batch-0 key blocks split across both queues so they land in parallel
        k_sb0 = setup.tile([128, 4, KS], F32, tag="k_sb0", bufs=1)
        k0_view = k_d[0].rearrange("(kb p) d -> p kb d", p=128)
        ident = consts.tile([128, 128], F32)
        # half-block granularity, alternating queues, so the first transpose
        # can start as early as possible and the rest stream in behind it
        nc.sync.dma_start(out=k_sb0[:, 0, 0:128], in_=k0_view[:, 0, 0:128])
        nc.gpsimd.dma_start(out=ident, in_=ident_d[:, :])
        nc.gpsimd.dma_start(out=k_sb0[:, 0, 128:256], in_=k0_view[:, 0, 128:256])
        nc.sync.dma_start(out=k_sb0[:, 1, 0:128], in_=k0_view[:, 1, 0:128])
        nc.gpsimd.dma_start(out=k_sb0[:, 1, 128:256], in_=k0_view[:, 1, 128:256])
        nc.sync.dma_start(out=k_sb0[:, 2, 0:128], in_=k0_view[:, 2, 0:128])
        nc.gpsimd.dma_start(out=k_sb0[:, 2, 128:256], in_=k0_view[:, 2, 128:256])
        nc.sync.dma_start(out=k_sb0[:, 3, 0:128], in_=k0_view[:, 3, 0:128])
        nc.gpsimd.dma_start(out=k_sb0[:, 3, 128:256], in_=k0_view[:, 3, 128:256])
        q_sb0 = setup.tile([NQ, QS], F32, tag="q_sb0", bufs=1)
        nc.sync.dma_start(out=q_sb0, in_=q_d[0])
        wq_sb = setup.tile([128, 2, H], F32, tag="wq_f", bufs=1)
        nc.sync.dma_start(out=wq_sb, in_=wq_d.rearrange("(kt p) m -> p kt m", p=128))
        wk_sb = setup.tile([128, 2, H], F32, tag="wk_f", bufs=1)
        nc.gpsimd.dma_start(out=wk_sb, in_=wk_d.rearrange("(kt p) m -> p kt m", p=128))
        wv_col = consts.tile([128, 2], F32)
        nc.gpsimd.dma_start(out=wv_col, in_=wv_d.rearrange("(t p) -> p t", p=128))
        diag_bf = consts.tile([128, NQ, NQ], BF16)
        nc.sync.dma_start(out=diag_bf, in_=diag_d[None, :].partition_broadcast(128))
        identb = consts.tile([128, 128], BF16)
        nc.gpsimd.dma_start(out=identb, in_=identb_d[:, :])
        k_sb1 = setup.tile([128, 4, KS], F32, tag="k_sb1", bufs=1)
        for kb in range(4):
            nc.sync.dma_start(
                out=k_sb1[:, kb],
                in_=k_d[1].rearrange("(kb p) d -> p kb d", p=128)[:, kb],
            )
        q_sb1 = setup.tile([NQ, QS], F32, tag="q_sb1", bufs=1)
        nc.sync.dma_start(out=q_sb1, in_=q_d[1])
        krow = consts.tile([128, NK], F32)
        nc.sync.dma_start(out=krow, in_=krow_d[None, :].partition_broadcast(128))
        v_sbs, valid_sbs = [], []
        for b in range(BPC):
            v_sb = setup.tile([128, 4, VD], F32, tag=f"v_sb{b}", name=f"v_sb{b}", bufs=1)
            nc.gpsimd.dma_start(
                out=v_sb, in_=v_d[b].rearrange("(kb p) d -> p kb d", p=128)
            )
            v_sbs.append(v_sb)
            valid_sb = setup.tile([128, 1], I32, tag=f"valid{b}", name=f"valid{b}")
            nc.gpsimd.dma_start(
                out=valid_sb, in_=vl_d[b : b + 1, :].partition_broadcast(128)
            )
            valid_sbs.append(valid_sb)

        k_sbs = [k_sb0, k_sb1]
        q_sbs = [q_sb0, q_sb1]

        # projection weights to bf16 (first on the DVE stream; their DMAs
        # are early on the gpsimd queue)
        wq_bf = consts.tile([128, 2, H], BF16)
        wk_bf = consts.tile([128, 2, H], BF16)
        for kt in range(2):
            nc.vector.tensor_copy(out=wk_bf[:, kt], in_=wk_sb[:, kt])
            nc.vector.tensor_copy(out=wq_bf[:, kt], in_=wq_sb[:, kt])

        onehot = consts.tile([128, 2, NQ, NQ], BF16)

        def setup_batch(b):
            """transposes + projections for batch b -> (kfT_bf, qfT_f32)"""
            k_sb, q_sb = k_sbs[b], q_sbs[b]
            kT_bf = setup.tile([128, 2, NK], BF16, tag="kT", name=f"kT{b}")
            for kb in range(4):
                for kt in range(2):
                    pst = ps_misc.tile(
                        [128, 512], F32, tag="ps_misc", name="pst_k"
                    )
                    nc.tensor.transpose(
                        pst[:, 0:128], k_sb[:, kb, kt * 128 : (kt + 1) * 128], ident
                    )
                    nc.vector.tensor_copy(
                        out=kT_bf[:, kt, kb * 128 : (kb + 1) * 128], in_=pst[:, 0:128]
                    )
            qT_bf = setup.tile([128, 2, NQ], BF16, tag="qT", name=f"qT{b}")
            for kt in range(2):
                pst = ps_misc.tile([128, 512], F32, tag="ps_misc", name="pst_q")
                nc.tensor.transpose(
                    pst[:, 0:NQ], q_sb[:, kt * 128 : (kt + 1) * 128], ident[0:NQ, 0:NQ]
                )
                nc.vector.tensor_copy(out=qT_bf[:, kt, :], in_=pst[:, 0:NQ])

            kfT_bf = setup.tile([128, 2, NK], BF16, tag="kfT", name=f"kfT{b}")
            for mt in range(2):
                psp = ps_misc.tile([128, 512], F32, tag="ps_misc", name="psp_k")
                for kt in range(2):
                    nc.tensor.matmul(
                        psp,
                        lhsT=wk_bf[:, kt, mt * 128 : (mt + 1) * 128],
                        rhs=kT_bf[:, kt, :],
                        start=(kt == 0),
                        stop=(kt == 1),
                    )
                if b == 0:
                    # ACT is idle before the first tanh slab: evacuate there
                    # to shorten the DVE critical path into the first adds
                    nc.scalar.copy(out=kfT_bf[:, mt], in_=psp)
                else:
                    nc.vector.tensor_copy(out=kfT_bf[:, mt], in_=psp)
            qfT_f32 = setup.tile([128, 2, NQ], F32, tag="qfTf", name=f"qfT{b}")
            for mt in range(2):
                psp = ps_misc.tile([128, 512], F32, tag="ps_misc", name="psp_q")
                for kt in range(2):
                    nc.tensor.matmul(
                        psp[:, 0:NQ],
                        lhsT=wq_bf[:, kt, mt * 128 : (mt + 1) * 128],
                        rhs=qT_bf[:, kt, :],
                        start=(kt == 0),
                        stop=(kt == 1),
                    )
                nc.vector.tensor_copy(out=qfT_f32[:, mt], in_=psp[:, 0:NQ])
            return kfT_bf, qfT_f32

        def feature_loop(b, kfT_bf, qfT_f32, sc_ps, build_onehot):
            chunks = CHUNKS_B0 if b == 0 else CHUNKS_B1
            first = [True]
            q0 = 0
            for ci, qn in enumerate(chunks):
                feat = slabs.tile([128, 12, 2, NK], BF16, tag="feat", name="feat")
                sum_bf = slabs.tile([128, 12, 2, NK], BF16, tag="sum", name="sum")
                for qi in range(qn):
                    q = q0 + qi
                    for ht in range(2):
                        nc.vector.tensor_scalar_add(
                            out=sum_bf[:, qi, ht],
                            in0=kfT_bf[:, ht],
                            scalar1=qfT_f32[:, ht, q : q + 1],
                        )
                nc.scalar.activation(
                    out=feat[:, 0:qn], in_=sum_bf[:, 0:qn], func=AF.Tanh
                )
                if build_onehot and ci == 0:
                    for ht in range(2):
                        nc.vector.tensor_scalar_mul(
                            out=onehot[:, ht],
                            in0=diag_bf,
                            scalar1=wv_col[:, ht : ht + 1],
                        )
                for qi in range(qn):
                    q = q0 + qi
                    for ht in range(2):
                        nc.tensor.matmul(
                            sc_ps[0:NQ],
                            lhsT=onehot[:, ht, q],
                            rhs=feat[:, qi, ht],
                            start=first[0],
                            stop=(ci == len(chunks) - 1 and qi == qn - 1 and ht == 1),
                        )
                        first[0] = False
                q0 += qn

        def finish_batch(b, sc_ps):
            # all tensors stay 128-row (rows 32g+o, o<16 are real queries,
            # the rest benign zero-score rows); free-dim-paced engine cost
            # is identical and the layout stays partition-aligned
            valid_f = setup.tile([128, 1], F32, tag="validf", name=f"vf{b}")
            nc.vector.tensor_copy(out=valid_f, in_=valid_sbs[b])
            bias_b = setup.tile([128, NK], F32, tag="bias", name=f"bias{b}")
            nc.vector.tensor_scalar(
                out=bias_b, in0=krow, scalar1=valid_f[:, 0:1], scalar2=None,
                op0=ALU.is_lt,
            )
            nc.vector.tensor_scalar(
                out=bias_b, in0=bias_b, scalar1=1.0, scalar2=-MASK_NEG,
                op0=ALU.subtract, op1=ALU.mult,
            )
            sc_sb = sm.tile([NQ, NK], F32, tag=f"scsb{b}", name=f"scsb{b}")
            nc.vector.tensor_tensor(
                out=sc_sb, in0=sc_ps[0:NQ], in1=bias_b[0:NQ], op=ALU.add
            )
            e_sb = sm.tile([NQ, NK], F32, tag=f"e{b}", name=f"e{b}")
            denom = sm.tile([NQ, 1], F32, tag=f"den{b}", name=f"den{b}")
            nc.scalar.activation(out=e_sb, in_=sc_sb, func=AF.Exp, accum_out=denom)
            recip = sm.tile([NQ, 1], F32, tag=f"rec{b}", name=f"rec{b}")
            nc.vector.reciprocal(recip, denom)
            attn = sm.tile([NQ, NK], BF16, tag=f"at{b}", name=f"at{b}")
            nc.vector.tensor_scalar_mul(out=attn, in0=e_sb, scalar1=recip[:, 0:1])
            v_bf = outp.tile([128, 4, VD], BF16, tag="v_bf", name=f"v_bf{b}")
            for kb in range(4):
                nc.vector.tensor_copy(out=v_bf[:, kb], in_=v_sbs[b][:, kb])

            attnT = outp.tile([128, 4, NQ], BF16, tag="attnT", name=f"attnT{b}")
            for kb in range(4):
                pst = ps_misc.tile(
                    [128, 1024], BF16, tag="ps_misc_b", name="pst_a"
                )
                nc.tensor.transpose(
                    pst[:, 0:NQ],
                    attn[:, kb * 128 : (kb + 1) * 128],
                    identb[0:NQ, 0:NQ],
                )
                nc.vector.tensor_copy(out=attnT[:, kb], in_=pst[:, 0:NQ])

            po = ps_out.tile([NQ, VD], F32, tag="po", name=f"po{b}")
            for kb in range(4):
                nc.tensor.matmul(
                    po,
                    lhsT=attnT[:, kb],
                    rhs=v_bf[:, kb],
                    start=(kb == 0),
                    stop=(kb == 3),
                )
            o_sb = outp.tile([NQ, VD], F32, tag="o_sb", name=f"o_sb{b}")
            nc.vector.tensor_copy(out=o_sb, in_=po)
            nc.gpsimd.dma_start(out=out_d[b], in_=o_sb)

        interleave = os.environ.get("ATTN_INTERLEAVE", "1") == "1"
        # batch 0 setup + loop
        kfT0, qfT0 = setup_batch(0)
        sc_ps0 = ps_sc.tile([128, NK], F32, tag="sc", name="sc0")
        feature_loop(0, kfT0, qfT0, sc_ps0, build_onehot=True)
        if interleave:
            # batch 1 setup emitted before batch 0's output chain so the
            # engine streams don't block behind the exp dependency
            kfT1, qfT1 = setup_batch(1)
            finish_batch(0, sc_ps0)
        else:
            finish_batch(0, sc_ps0)
            kfT1, qfT1 = setup_batch(1)
        sc_ps1 = ps_sc.tile([128, NK], F32, tag="sc", name="sc1")
        feature_loop(1, kfT1, qfT1, sc_ps1, build_onehot=False)
        finish_batch(1, sc_ps1)

    nc.compile()
    return nc


_NC_CACHE = None
LAST_RESULTS = None


def kernel(queries, keys, values, valid_lens, W_q, W_k, w_v):
    global _NC_CACHE, LAST_RESULTS
    if _NC_CACHE is None:
        _NC_CACHE = _build()
    nc = _NC_CACHE

    queries = np.ascontiguousarray(queries, dtype=np.float32)
    keys = np.ascontiguousarray(keys, dtype=np.float32)
    values = np.ascontiguousarray(values, dtype=np.float32)
    valid_lens = np.ascontiguousarray(valid_lens, dtype=np.int32)
    W_q = np.ascontiguousarray(W_q, dtype=np.float32)
    W_k = np.ascontiguousarray(W_k, dtype=np.float32)
    w_v = np.ascontiguousarray(w_v, dtype=np.float32)

    in_maps = []
    for c in range(NCORES):
        lo, hi = c * BPC, (c + 1) * BPC
        in_maps.append(
            {
                "queries": queries[lo:hi],
                "keys": keys[lo:hi],
                "values": values[lo:hi],
                "valid_lens": valid_lens[lo:hi].reshape(BPC, 1),
                "W_q": W_q,
                "W_k": W_k,
                "w_v": w_v,
            }
        )

    trace = os.environ.get("ATTN_TRACE", "0") == "1"
    res = run_bass_kernel_spmd(
        nc, in_maps, core_ids=list(range(NCORES)), trace=trace
    )
    LAST_RESULTS = res
    return np.concatenate([r["out"] for r in res.results], axis=0)
